# revision 33
# baseline (speedup 1.0000x reference)
"""Causal attention (B=4, S=4096, D=64, fp32) on 8 Trainium2 NeuronCores.

Strategy
--------
Sharding: 2 cores per batch element; the two cores of a batch split the KV
blocks by parity (even / odd 128-row blocks). Each core computes, for every
query position of its batch, the *unnormalized* attention numerator and the
softmax denominator contribution of its own KV half. The host sums the two
halves and divides (exactly linear, since the softmax uses no max-subtraction:
scores/8 are bounded by ~|6| for N(0,1) inputs, so exp never overflows fp32).

Per-core device kernel (identical SPMD program; per-core behavior comes only
from input data):
  - scores^T layout: S_T[kv, q] = K @ Q^T, computed as
    matmul(lhsT=K^T block [64,128], rhs=Q^T tile [64,512]) in fp32r
    (1 cycle/row on the PE; ~1.6e-4 rel err).
  - causal masking: within a 512-wide q tile only the last two parity-KV
    blocks straddle the diagonal. Two per-core mask tiles (input data) are
    added to the scores of exactly those two loop positions, making the
    program core-independent.
  - softmax: P = exp(scores/8 + mask/8) on the ACT engine (scale=0.125
    applied by the activation's free affine; masked entries become
    exp(-1.25e9) = 0 exactly).
  - numerator+denominator: matmul(lhsT=[V | 1] block [128,65], rhs=P
    [128,512]) accumulated over KV blocks in PSUM; row 64 is sum(P) = the
    softmax denominator. Padded key positions are handled by zeroing their V
    rows AND their ones-column entry on the host: they then contribute 0 to
    both numerator and denominator (exact).
Host: transposes Q/K (device PE/DVE transposes are expensive; layout prep is
part of sharding), packs per-core inputs, and combines/normalizes/transposes
the outputs.
"""

import numpy as np
from contextlib import ExitStack

import concourse.tile as tile
from concourse import bacc, mybir
from concourse.bass_utils import run_bass_kernel_spmd

B, S, D = 4, 4096, 64
NCORES = 8
BLK = 128            # kv block rows
QTW = 512            # q tile width
NQT = S // QTW       # 8 q tiles
PAR = S // BLK // 2  # 16 kv blocks per parity half
WARMUP_MMS = 8       # dummy matmuls to open the PE HAM clock gate at startup
NEG = np.float32(-1e10)

_prog_cache = {}


def _build_program():
    if "nc" in _prog_cache:
        return _prog_cache["nc"]
    nc = bacc.Bacc("TRN2", target_bir_lowering=False, debug=False, num_devices=NCORES)
    f32, f16 = mybir.dt.float32, mybir.dt.float16
    Exp = mybir.ActivationFunctionType.Exp

    # Q^T / K^T duplicated onto partitions 64-127 so two K=64 matmuls can run
    # concurrently in the PE array via row tiling (tile_position).
    qt_d = nc.dram_tensor("qt", [2 * D, S], f16, kind="ExternalInput").ap()
    kt_d = nc.dram_tensor("kt", [2 * D, PAR * BLK], f16, kind="ExternalInput").ap()
    vp_d = nc.dram_tensor("vp", [BLK, PAR * 65], f16, kind="ExternalInput").ap()
    mk_d = nc.dram_tensor("mk", [BLK, 2 * QTW], f16, kind="ExternalInput").ap()
    out_d = nc.dram_tensor("out", [65, S], f32, kind="ExternalOutput").ap()

    with tile.TileContext(nc) as tc, ExitStack() as ctx:
        const = ctx.enter_context(tc.tile_pool(name="const", bufs=1))
        ppool = ctx.enter_context(tc.tile_pool(name="pp", bufs=2))
        opool = ctx.enter_context(tc.tile_pool(name="op", bufs=2))
        sc_ps = ctx.enter_context(tc.tile_pool(name="scps", bufs=1, space="PSUM"))
        out_ps = ctx.enter_context(tc.tile_pool(name="ops", bufs=2, space="PSUM"))

        # Input DMAs split across both HWDGE rings (sync + scalar) so issue
        # overhead (~0.7us each, FIFO per ring) doesn't serialize; ordered so
        # q-tile 0's working set (qt0, kt[0:256], vp blocks 0-1, masks) lands
        # first on each ring.
        mk_s = const.tile([BLK, 2 * QTW], f16)
        kt_s = const.tile([2 * D, PAR * BLK], f16)
        vp_s = const.tile([BLK, PAR * 65], f16)
        qt_s = const.tile([2 * D, S], f16)
        nc.scalar.dma_start(kt_s[:, 0:1024], kt_d[:, 0:1024])
        nc.gpsimd.dma_start(vp_s[:], vp_d[:])
        nc.scalar.dma_start(kt_s[:, 1024:], kt_d[:, 1024:])
        nc.scalar.dma_start(mk_s[:], mk_d[:])
        for t in range(NQT - 1, -1, -1):
            nc.sync.dma_start(qt_s[:, t * QTW : (t + 1) * QTW], qt_d[:, t * QTW : (t + 1) * QTW])

        # PE warmup: the HAM clock gate keeps the PE at 1.2 GHz until it has
        # been busy ~3.4us. The input DMAs take ~4us after the ~6us NEFF
        # preamble, so run dependency-free dummy matmuls in that window to
        # reach 2.4 GHz before the first real matmul (and keep the window
        # busy right up to the handoff).
        wsrc = const.tile([BLK, QTW], f16, name="wsrc")
        nc.vector.memset(wsrc[:], 0.0)
        wps = sc_ps.tile([BLK, 4 * QTW], f32, tag="scA", name="wps")
        for _ in range(WARMUP_MMS):
            nc.tensor.matmul(wps[:, 0:QTW], wsrc[:, 0:BLK], wsrc[:], start=True, stop=True)

        # Tiles deepest-first: tile 7's 8 back-to-back pairs absorb the
        # pipeline ramp; within each tile ascending kv order, the diagonal
        # (masked, column-narrowed) pair last.
        #
        # Boundary pair layout in sc/pt: [block lo: full 512 cols | block
        # lo+1: cols 256:512 only] — columns < 256 of the last kv block are
        # entirely above the diagonal for both parities, so they are neither
        # computed nor exp'd.
        #
        # Exp slots alternate between a 4-bank (2 pairs) and a 2-bank (1 pair)
        # PSUM buffer — 24 ACT ops instead of 36 amortizes the per-op access
        # latency while fitting the 8-bank PSUM alongside the 2 output banks.
        flat = []
        for T in range(NQT - 1, -1, -1):
            depth = 2 * T + 2
            for lo in list(range(0, depth - 2, 2)) + [depth - 2]:
                flat.append((T, lo, lo == depth - 2))

        slots = []
        i = 0
        cap = 2
        while i < len(flat):
            n = min(cap, len(flat) - i)
            slots.append(flat[i : i + n])
            i += n
            cap = 3 - cap  # alternate 2, 1, 2, 1, ...

        ops_tiles = {}
        mm2_count = {}
        for si, slot in enumerate(slots):
            two = len(slot) == 2
            sc = sc_ps.tile(
                [BLK, (4 if two else 2) * QTW], f32,
                tag="scA" if two else "scB", name=f"sc{si}",
            )
            pt = ppool.tile(
                [BLK, (4 if two else 2) * QTW], f16,
                tag="ptA" if two else "ptB", name=f"pt{si}",
            )
            ew = 0
            for p, (T, lo, boundary) in enumerate(slot):
                if T not in ops_tiles:
                    ops_tiles[T] = out_ps.tile([65, QTW], f32, tag="ops", name=f"ops{T}")
                    mm2_count[T] = 0
                base = 2 * QTW * p
                wid = (QTW, QTW // 2) if boundary else (QTW, QTW)
                for k, rg in ((0, 0), (1, D)):  # row group 0 / 64
                    blk = lo + k
                    nc.tensor.matmul(
                        sc[:, base + k * QTW : base + k * QTW + wid[k]],
                        kt_s[rg : rg + D, blk * BLK : (blk + 1) * BLK],
                        qt_s[rg : rg + D, T * QTW + (QTW - wid[k]) : (T + 1) * QTW],
                        start=True,
                        stop=True,
                        tile_position=(rg, 0),
                    )
                ew = base + QTW + wid[1]
            nc.scalar.activation(pt[:, 0:ew], sc[:, 0:ew], Exp, scale=0.125)
            for p, (T, lo, boundary) in enumerate(slot):
                base = 2 * QTW * p
                wid = (QTW, QTW // 2) if boundary else (QTW, QTW)
                if boundary:
                    # Multiplicative causal mask (0/1) on P after exp: keeps
                    # masking off the ACT critical path (PE absorbs it).
                    nc.vector.tensor_mul(
                        pt[:, base : base + QTW], pt[:, base : base + QTW], mk_s[:, 0:QTW]
                    )
                    nc.vector.tensor_mul(
                        pt[:, base + QTW : base + QTW + wid[1]],
                        pt[:, base + QTW : base + QTW + wid[1]],
                        mk_s[:, QTW + QTW // 2 : 2 * QTW],
                    )
                ops = ops_tiles[T]
                depth = 2 * T + 2
                for k in range(2):
                    blk = lo + k
                    mm2_count[T] += 1
                    nc.tensor.matmul(
                        ops[:, QTW - wid[k] : QTW],
                        vp_s[:, blk * 65 : (blk + 1) * 65],
                        pt[:, base + k * QTW : base + k * QTW + wid[k]],
                        start=(mm2_count[T] == 1),
                        stop=(mm2_count[T] == depth),
                    )
                if mm2_count[T] == depth:
                    osb = opool.tile([65, QTW], f32, tag="osb", name=f"osb{T}")
                    nc.vector.tensor_copy(osb[:], ops[:])
                    nc.sync.dma_start(out_d[:, T * QTW : (T + 1) * QTW], osb[:])
                    del ops_tiles[T]

    nc.compile()
    _prog_cache["nc"] = nc
    return nc


def _make_masks(h):
    """[128, 1024] fp16 multiplicative (1=keep, 0=masked) masks: two stacked
    tiles for the 2nd-to-last / last parity-kv loop positions of every q tile
    (relative diagonal offsets r = h and r = h + 2)."""
    tri = (np.arange(QTW)[None, :BLK] >= np.arange(BLK)[:, None]).astype(np.float16)
    full = np.zeros((BLK, BLK), dtype=np.float16)  # fully masked block
    keep = np.ones((BLK, BLK), dtype=np.float16)

    def mask_for_r(r):
        cols = []
        for cb in range(QTW // BLK):
            if cb < r:
                cols.append(full)
            elif cb == r:
                cols.append(tri)
            else:
                cols.append(keep)
        return np.concatenate(cols, axis=1)  # [128, 512]

    return np.concatenate([mask_for_r(h), mask_for_r(h + 2)], axis=1)


def kernel(query, key, value, padding):
    query = np.asarray(query, dtype=np.float32)
    key = np.asarray(key, dtype=np.float32)
    value = np.asarray(value, dtype=np.float32)
    padding = np.asarray(padding, dtype=bool)

    nc = _build_program()

    in_maps = []
    for c in range(NCORES):
        b, h = divmod(c, 2)
        qt1 = np.ascontiguousarray(query[b].T).astype(np.float16)  # [64, 4096]
        qt = np.concatenate([qt1, qt1], axis=0)  # [128, 4096] (row-tiling dup)
        kT = key[b].T  # [64, 4096] view
        blocks = [2 * i + h for i in range(PAR)]
        kt = np.concatenate([kT[:, BLK * j : BLK * (j + 1)] for j in blocks], axis=1)
        kt1 = np.ascontiguousarray(kt).astype(np.float16)  # [64, 2048]
        kt = np.concatenate([kt1, kt1], axis=0)  # [128, 2048] (row-tiling dup)
        vp = np.zeros((BLK, PAR * 65), dtype=np.float16)
        for i, j in enumerate(blocks):
            vblk = value[b, BLK * j : BLK * (j + 1), :].copy()
            pblk = padding[b, BLK * j : BLK * (j + 1)]
            vblk[pblk] = 0.0
            vp[:, 65 * i : 65 * i + 64] = vblk
            vp[:, 65 * i + 64] = np.where(pblk, 0.0, 1.0)
        in_maps.append({"qt": qt, "kt": kt, "vp": vp, "mk": _make_masks(h)})

    global _last_in_maps
    _last_in_maps = in_maps
    res = run_bass_kernel_spmd(nc, in_maps, list(range(NCORES)))

    out = np.empty((B, S, D), dtype=np.float32)
    for b in range(B):
        r0 = res.results[2 * b]["out"].astype(np.float64)
        r1 = res.results[2 * b + 1]["out"].astype(np.float64)
        num = r0[:64] + r1[:64]  # [64, 4096]
        den = r0[64] + r1[64]  # [4096]
        out[b] = (num / den).T.astype(np.float32)
    return out


# revision 34
# speedup vs baseline: 1.6055x; 1.6055x over previous
"""Causal attention (B=4, S=4096, D=64, fp32) on 8 Trainium2 NeuronCores.

Strategy
--------
Sharding: 2 cores per batch element; the two cores of a batch split the KV
blocks by parity (even / odd 128-row blocks). Each core computes, for every
query position of its batch, the *unnormalized* attention numerator and the
softmax denominator contribution of its own KV half. The host sums the two
halves and divides (exactly linear, since the softmax uses no max-subtraction:
scores/8 are bounded by ~|6| for N(0,1) inputs, so exp never overflows fp32).

Per-core device kernel (identical SPMD program; per-core behavior comes only
from input data):
  - scores^T layout: S_T[kv, q] = K @ Q^T, computed as
    matmul(lhsT=K^T block [64,128], rhs=Q^T tile [64,512]) in fp32r
    (1 cycle/row on the PE; ~1.6e-4 rel err).
  - causal masking: within a 512-wide q tile only the last two parity-KV
    blocks straddle the diagonal. Two per-core mask tiles (input data) are
    added to the scores of exactly those two loop positions, making the
    program core-independent.
  - softmax: P = exp(scores/8 + mask/8) on the ACT engine (scale=0.125
    applied by the activation's free affine; masked entries become
    exp(-1.25e9) = 0 exactly).
  - numerator+denominator: matmul(lhsT=[V | 1] block [128,65], rhs=P
    [128,512]) accumulated over KV blocks in PSUM; row 64 is sum(P) = the
    softmax denominator. Padded key positions are handled by zeroing their V
    rows AND their ones-column entry on the host: they then contribute 0 to
    both numerator and denominator (exact).
Host: transposes Q/K (device PE/DVE transposes are expensive; layout prep is
part of sharding), packs per-core inputs, and combines/normalizes/transposes
the outputs.
"""

import numpy as np
from contextlib import ExitStack

import concourse.tile as tile
from concourse import bacc, mybir
from concourse.bass_utils import run_bass_kernel_spmd

B, S, D = 4, 4096, 64
NCORES = 8
BLK = 128            # kv block rows
QTW = 512            # q tile width
NQT = S // QTW       # 8 q tiles
PAR = S // BLK // 2  # 16 kv blocks per parity half
WARMUP_MMS = 8       # dummy matmuls to open the PE HAM clock gate at startup
NEG = np.float32(-1e10)

_prog_cache = {}


def _build_program():
    if "nc" in _prog_cache:
        return _prog_cache["nc"]
    nc = bacc.Bacc("TRN2", target_bir_lowering=False, debug=False, num_devices=NCORES)
    f32, f16 = mybir.dt.float32, mybir.dt.float16
    Exp = mybir.ActivationFunctionType.Exp

    # Q^T / K^T duplicated onto partitions 64-127 so two K=64 matmuls can run
    # concurrently in the PE array via row tiling (tile_position).
    qt_d = nc.dram_tensor("qt", [2 * D, S], f16, kind="ExternalInput").ap()
    kt_d = nc.dram_tensor("kt", [2 * D, PAR * BLK], f16, kind="ExternalInput").ap()
    vp_d = nc.dram_tensor("vp", [BLK, PAR * 65], f16, kind="ExternalInput").ap()
    mk_d = nc.dram_tensor("mk", [BLK, 2 * QTW], f16, kind="ExternalInput").ap()
    out_d = nc.dram_tensor("out", [65, S], f32, kind="ExternalOutput").ap()

    with tile.TileContext(nc) as tc, ExitStack() as ctx:
        const = ctx.enter_context(tc.tile_pool(name="const", bufs=1))
        ppool = ctx.enter_context(tc.tile_pool(name="pp", bufs=3))
        opool = ctx.enter_context(tc.tile_pool(name="op", bufs=2))
        sc_ps = ctx.enter_context(tc.tile_pool(name="scps", bufs=3, space="PSUM"))
        out_ps = ctx.enter_context(tc.tile_pool(name="ops", bufs=2, space="PSUM"))

        # Input DMAs split across both HWDGE rings (sync + scalar) so issue
        # overhead (~0.7us each, FIFO per ring) doesn't serialize; ordered so
        # q-tile 0's working set (qt0, kt[0:256], vp blocks 0-1, masks) lands
        # first on each ring.
        mk_s = const.tile([BLK, 2 * QTW], f16)
        kt_s = const.tile([2 * D, PAR * BLK], f16)
        vp_s = const.tile([BLK, PAR * 65], f16)
        qt_s = const.tile([2 * D, S], f16)
        nc.scalar.dma_start(kt_s[:, 0:1024], kt_d[:, 0:1024])
        nc.gpsimd.dma_start(vp_s[:], vp_d[:])
        nc.scalar.dma_start(kt_s[:, 1024:], kt_d[:, 1024:])
        nc.scalar.dma_start(mk_s[:], mk_d[:])
        for t in range(NQT - 1, -1, -1):
            nc.sync.dma_start(qt_s[:, t * QTW : (t + 1) * QTW], qt_d[:, t * QTW : (t + 1) * QTW])

        # PE warmup: the HAM clock gate keeps the PE at 1.2 GHz until it has
        # been busy ~3.4us. The input DMAs take ~4us after the ~6us NEFF
        # preamble, so run dependency-free dummy matmuls in that window to
        # reach 2.4 GHz before the first real matmul (and keep the window
        # busy right up to the handoff).
        wsrc = const.tile([BLK, QTW], f16, name="wsrc")
        nc.vector.memset(wsrc[:], 0.0)
        wps = sc_ps.tile([BLK, 2 * QTW], f32, tag="sc", name="wps")
        for _ in range(WARMUP_MMS):
            nc.tensor.matmul(wps[:, 0:QTW], wsrc[:, 0:BLK], wsrc[:], start=True, stop=True)

        # Tiles deepest-first: tile 7's 8 back-to-back pairs absorb the
        # pipeline ramp (no tile-boundary serialization while the PE warms and
        # ACT builds backlog). Within the first tile the diagonal (masked)
        # pair goes last (no ACT backlog exists yet to hide behind); in later
        # tiles it goes first so its post-exp DVE mask-muls overlap ACT work.
        #
        # Boundary pair layout in sc/pt: [block lo: full 512 cols | block
        # lo+1: cols 256:512 only] - columns < 256 of the last kv block are
        # entirely above the diagonal for both parities, so they are neither
        # computed nor exp'd (the exp covers [0:768) contiguously).
        for ti, T in enumerate(range(NQT - 1, -1, -1)):
            depth = 2 * T + 2  # parity kv blocks covering this q tile (even)
            body = list(range(0, depth - 2, 2))
            pair_lo = body + [depth - 2] if ti == 0 else [depth - 2] + body
            ops = out_ps.tile([65, QTW], f32, tag="ops", name=f"ops{T}")
            n_mm2 = 0
            for pi, lo in enumerate(pair_lo):
                boundary = lo == depth - 2
                sc = sc_ps.tile([BLK, 2 * QTW], f32, tag="sc")
                wid = (QTW, QTW // 2) if boundary else (QTW, QTW)
                for k, rg in ((0, 0), (1, D)):  # row group 0 / 64
                    blk = lo + k
                    nc.tensor.matmul(
                        sc[:, k * QTW : k * QTW + wid[k]],
                        kt_s[rg : rg + D, blk * BLK : (blk + 1) * BLK],
                        qt_s[rg : rg + D, T * QTW + (QTW - wid[k]) : (T + 1) * QTW],
                        start=True,
                        stop=True,
                        tile_position=(rg, 0),
                    )
                pt = ppool.tile([BLK, 2 * QTW], f16, tag="pt")
                ew = QTW + wid[1]
                nc.scalar.activation(pt[:, 0:ew], sc[:, 0:ew], Exp, scale=0.125)
                if boundary:
                    # Multiplicative causal mask (0/1) on P after exp: keeps
                    # masking off the ACT critical path (PE absorbs it).
                    nc.vector.tensor_mul(pt[:, 0:QTW], pt[:, 0:QTW], mk_s[:, 0:QTW])
                    nc.vector.tensor_mul(
                        pt[:, QTW:ew], pt[:, QTW:ew], mk_s[:, QTW + QTW // 2 : 2 * QTW]
                    )
                for k in ((1, 0) if boundary and ti == 0 else (0, 1)):
                    blk = lo + k
                    n_mm2 += 1
                    nc.tensor.matmul(
                        ops[:, QTW - wid[k] : QTW],
                        vp_s[:, blk * 65 : (blk + 1) * 65],
                        pt[:, k * QTW : k * QTW + wid[k]],
                        start=(n_mm2 == 1),
                        stop=(n_mm2 == depth),
                    )
            osb = opool.tile([65, QTW], f32, tag="osb", name=f"osb{T}")
            nc.vector.tensor_copy(osb[:], ops[:])
            nc.sync.dma_start(out_d[:, T * QTW : (T + 1) * QTW], osb[:])

    nc.compile()
    _prog_cache["nc"] = nc
    return nc


def _make_masks(h):
    """[128, 1024] fp16 multiplicative (1=keep, 0=masked) masks: two stacked
    tiles for the 2nd-to-last / last parity-kv loop positions of every q tile
    (relative diagonal offsets r = h and r = h + 2)."""
    tri = (np.arange(QTW)[None, :BLK] >= np.arange(BLK)[:, None]).astype(np.float16)
    full = np.zeros((BLK, BLK), dtype=np.float16)  # fully masked block
    keep = np.ones((BLK, BLK), dtype=np.float16)

    def mask_for_r(r):
        cols = []
        for cb in range(QTW // BLK):
            if cb < r:
                cols.append(full)
            elif cb == r:
                cols.append(tri)
            else:
                cols.append(keep)
        return np.concatenate(cols, axis=1)  # [128, 512]

    return np.concatenate([mask_for_r(h), mask_for_r(h + 2)], axis=1)


def kernel(query, key, value, padding):
    query = np.asarray(query, dtype=np.float32)
    key = np.asarray(key, dtype=np.float32)
    value = np.asarray(value, dtype=np.float32)
    padding = np.asarray(padding, dtype=bool)

    nc = _build_program()

    in_maps = []
    for c in range(NCORES):
        b, h = divmod(c, 2)
        qt1 = np.ascontiguousarray(query[b].T).astype(np.float16)  # [64, 4096]
        qt = np.concatenate([qt1, qt1], axis=0)  # [128, 4096] (row-tiling dup)
        kT = key[b].T  # [64, 4096] view
        blocks = [2 * i + h for i in range(PAR)]
        kt = np.concatenate([kT[:, BLK * j : BLK * (j + 1)] for j in blocks], axis=1)
        kt1 = np.ascontiguousarray(kt).astype(np.float16)  # [64, 2048]
        kt = np.concatenate([kt1, kt1], axis=0)  # [128, 2048] (row-tiling dup)
        vp = np.zeros((BLK, PAR * 65), dtype=np.float16)
        for i, j in enumerate(blocks):
            vblk = value[b, BLK * j : BLK * (j + 1), :].copy()
            pblk = padding[b, BLK * j : BLK * (j + 1)]
            vblk[pblk] = 0.0
            vp[:, 65 * i : 65 * i + 64] = vblk
            vp[:, 65 * i + 64] = np.where(pblk, 0.0, 1.0)
        in_maps.append({"qt": qt, "kt": kt, "vp": vp, "mk": _make_masks(h)})

    global _last_in_maps
    _last_in_maps = in_maps
    res = run_bass_kernel_spmd(nc, in_maps, list(range(NCORES)))

    out = np.empty((B, S, D), dtype=np.float32)
    for b in range(B):
        r0 = res.results[2 * b]["out"].astype(np.float64)
        r1 = res.results[2 * b + 1]["out"].astype(np.float64)
        num = r0[:64] + r1[:64]  # [64, 4096]
        den = r0[64] + r1[64]  # [4096]
        out[b] = (num / den).T.astype(np.float32)
    return out
